# revision 1
# baseline (speedup 1.0000x reference)
"""Trainium2 Bass kernel for the YOLO-style grid loss (nn_Loss_12326556139840).

Strategy: pure data parallel over 8 NeuronCores, 2048 batch rows each.

Host-side prep (not on the HW critical path): inputs are cast to fp16 and
re-arranged into *plane-major* layout — one [128, C] contiguous plane per
per-cell quantity (px0, py0, pw0, ph0, px1, ..., pc0, pc1, 20 class
channels; likewise for the target, whose two conf channels collapse to a
single obj plane).  Every device-side op is then a unit-stride fp16
vector op (2x/4x DVE perf modes), DMA is one contiguous 23.5KB chunk per
partition per tile, and there are no strided access patterns anywhere.

Device-side math (branchless, per cell):
  - iw = relu(min(pw+tw-2|px-tx|, 2*min(pw,tw)))  (2x-scaled overlap)
  - responsible box via cross-multiplied IoU compare (i1*u0 vs i0*u1)
  - new_conf folded to arithmetic on r = [iou1>iou0], n = [iou0>iou1]:
    conf target for resp box = r+n, for non-resp box = 1-(r+n)
  - fxy = frac(x*7) with frac<=0 -> +1, via the fp16 +1024 rounding trick
  - all loss weights folded as sqrt(w) into the residual planes; one
    ACT Square+accum_out per group reduces everything to [128,1] partials.

Work split: DVE does the vector algebra (selects are arithmetic blends
a0 + m*(a1-a0); relu is a tensor_scalar max), ScalarE (ACT) does abs/sqrt
and the square+accum reductions, GPSIMD does the class-plane obj masking.
The class planes live in their own DRAM tensor with pre/tgt interleaved
in [5p|5t] chunks so each chunk is ONE ~1MB DMA; the class path is
chunked 4-way per group and emitted first so its DMA, GPSIMD masking and
two-half ACT squares pipeline under the box/conf math.  The IoU compare
uses i1*A0 > i0*A1 (A = area sum; the i0*i1 union terms cancel), and
fxy uses fp16-storage rounding at the [1024,2048) grain-1.0 window.
Output: raw [128, 12] accumulator columns; the host does the final sum.
"""

import numpy as np

import concourse.bacc as bacc
import concourse.tile as tile
from concourse import mybir
from concourse.bass_utils import run_bass_kernel_spmd

F32 = mybir.dt.float32
F16 = mybir.dt.float16
Alu = mybir.AluOpType
Act = mybir.ActivationFunctionType

B = 16384
NCORES = 8
BPC = B // NCORES            # 2048 batch rows per core
P = 128                      # partitions
CELLS = BPC * 49             # 100352 cells per core
CPP = CELLS // P             # 784 cells per partition
G = 2                        # groups (DMA/compute pipeline stages)
C = CPP // G                 # 392 cells per partition per group

NPP = 30                     # pre planes:  x0 y0 w0 h0 x1 y1 w1 h1 c0 c1 cls*20
NPT = 29                     # tgt planes:  x0 y0 w0 h0 x1 y1 w1 h1 obj cls*20

EPS = 1e-7
MAGIC = 8388608.0            # 2^23: round-to-nearest in the DVE's fp32 ALU
SQRT5 = float(np.sqrt(5.0))
SQRTH = float(np.sqrt(0.5))

# channel picks from the raw [.., 49, 30] layout
PRE_CH = [0, 1, 2, 3, 5, 6, 7, 8, 4, 9]
TGT_CH = [0, 1, 2, 3, 5, 6, 7, 8, 4]
# class channels from concat(pre, tgt) [.., 60]: 4 chunks of (5 pre | 5 tgt)
CLS_CH = sum(
    [list(range(10 + 5 * ci, 15 + 5 * ci))
     + list(range(40 + 5 * ci, 45 + 5 * ci)) for ci in range(4)],
    [],
)

def _build(nloop: int = 1, variant: str = "full"):
    nc = bacc.Bacc()
    pre_d = nc.declare_dram_parameter("pre", [G, P, 10 * C], F16, isOutput=False)
    tgt_d = nc.declare_dram_parameter("tgt", [G, P, 9 * C], F16, isOutput=False)
    cls_d = nc.declare_dram_parameter("cls", [G, P, 40 * C], F16, isOutput=False)
    out_d = nc.declare_dram_parameter("out", [P, 6 * nloop * G], F32, isOutput=True)
    if variant == "debug":
        dbg_R = nc.declare_dram_parameter("dbg_R", [P, 32 * C], F16, isOutput=True)
        dbg_r = nc.declare_dram_parameter("dbg_r", [P, 2 * C], F16, isOutput=True)
        dbg_fxy = nc.declare_dram_parameter("dbg_fxy", [P, 2 * C], F16, isOutput=True)

    with tile.TileContext(nc) as tc:
        with (
            tc.tile_pool(name="pin", bufs=2) as pin,
            tc.tile_pool(name="tin", bufs=2) as tin,
            tc.tile_pool(name="res", bufs=2) as resp_,
            tc.tile_pool(name="w4", bufs=2) as w4,
            tc.tile_pool(name="w2", bufs=2) as w2,
            tc.tile_pool(name="w1", bufs=1) as w1,
            tc.tile_pool(name="acc", bufs=1) as accp,
        ):
            v = nc.vector
            s = nc.scalar
            g_ = nc.gpsimd

            NCOL = 6 * nloop * G
            acc32 = accp.tile([P, NCOL], F32, tag="acc")
            v.memset(acc32, 0.0)
            eps_b = accp.tile([P, 1], F32, tag="epsb")
            v.memset(eps_b, EPS)

            for rep in range(nloop):
                for gi in range(G):
                    col = rep * G + gi
                    ptile = pin.tile([P, 10, C], F16, tag="p")
                    ttile = tin.tile([P, 9, C], F16, tag="t")
                    ktile = pin.tile([P, 40, C], F16, tag="k")
                    pre_v = pre_d[gi].rearrange("p (q c) -> p q c", c=C)
                    tgt_v = tgt_d[gi].rearrange("p (q c) -> p q c", c=C)
                    cls_v = cls_d[gi].rearrange("p (q c) -> p q c", c=C)
                    nc.sync.dma_start(out=ptile, in_=pre_v)
                    nc.sync.dma_start(out=ttile, in_=tgt_v)
                    for ci in range(4):
                        nc.sync.dma_start(
                            out=ktile[:, 10 * ci : 10 * ci + 10, :],
                            in_=cls_v[:, 10 * ci : 10 * ci + 10, :],
                        )

                    # ---- class path first so GPSIMD + its square start early ----
                    R = resp_.tile([P, 32, C], F16, tag="R")
                    obj = ttile[:, 8, :]              # [P,C]
                    obj_b5 = obj.unsqueeze(1).broadcast_to([P, 5, C])
                    for ci in range(4):
                        pl = slice(12 + 5 * ci, 17 + 5 * ci)
                        v.tensor_sub(
                            R[:, pl, :],
                            ktile[:, 10 * ci : 10 * ci + 5, :],
                            ktile[:, 10 * ci + 5 : 10 * ci + 10, :],
                        )
                        g_.tensor_mul(R[:, pl, :], R[:, pl, :], obj_b5)
                        if ci == 1:
                            s.activation(
                                R[:, 12:22, :], R[:, 12:22, :], Act.Square,
                                accum_out=acc32[:, 6 * col : 6 * col + 1],
                            )
                    s.activation(
                        R[:, 22:32, :], R[:, 22:32, :], Act.Square,
                        accum_out=acc32[:, 6 * col + 1 : 6 * col + 2],
                    )

                    # box-structured views: [P, box, quant(x y w h), C]
                    pv = ptile[:, 0:8, :].rearrange("p (b q) c -> p b q c", b=2)
                    tv = ttile[:, 0:8, :].rearrange("p (b q) c -> p b q c", b=2)
                    pxy = pv[:, :, 0:2, :]
                    pwh = pv[:, :, 2:4, :]
                    txy = tv[:, :, 0:2, :]
                    twh = tv[:, :, 2:4, :]
                    pc01 = ptile[:, 8:10, :]          # [P,2,C]

                    # ---------------- IoU / responsibility ----------------
                    # three [P,2,2,C] scratch tiles, reused in-place
                    t_dd = w4.tile([P, 2, 2, C], F16, tag="dd")
                    t_ss = w4.tile([P, 2, 2, C], F16, tag="ss")
                    t_mm = w4.tile([P, 2, 2, C], F16, tag="mm")
                    v.tensor_sub(t_dd, pxy, txy)
                    s.activation(t_dd, t_dd, Act.Abs, scale=2.0)       # a2 = 2|d|
                    v.tensor_add(t_ss, pwh, twh)
                    v.tensor_sub(t_ss, t_ss, t_dd)                     # ee
                    v.tensor_tensor(t_mm, pwh, twh, op=Alu.min)
                    v.scalar_tensor_tensor(t_mm, t_mm, 2.0, t_ss, op0=Alu.mult, op1=Alu.min)
                    v.tensor_single_scalar(t_mm, t_mm, 0.0, op=Alu.max)  # iwr

                    i4 = w2.tile([P, 2, C], F16, tag="i4")
                    v.tensor_mul(i4, t_mm[:, :, 0, :], t_mm[:, :, 1, :])
                    ap_ = w2.tile([P, 2, C], F16, tag="ap")
                    v.tensor_mul(ap_, pv[:, :, 2, :], pv[:, :, 3, :])
                    at_ = w2.tile([P, 2, C], F16, tag="at")
                    v.tensor_mul(at_, tv[:, :, 2, :], tv[:, :, 3, :])
                    sa = ap_
                    v.tensor_add(sa, ap_, at_)
                    # iou1 > iou0  <=>  i1*u0 > i0*u1 with u_b = A_b - i_b
                    # (A = sum of box areas); the i0*i1 terms cancel, so
                    # comparing against A_b directly is exactly equivalent —
                    # the union is never needed.  The +4e-4 keeps the
                    # reference's eps tie-break (zero overlap -> smaller A,
                    # i.e. smaller union, wins).
                    ie = i4
                    v.tensor_scalar_add(ie, i4, 4e-4)
                    cr = w2.tile([P, 2, C], F16, tag="cr")
                    v.tensor_mul(cr, ie, sa[:, ::-1, :])
                    r = w1.tile([P, C], F16, tag="r")
                    v.tensor_tensor(r, cr[:, 1, :], cr[:, 0, :], op=Alu.is_gt)
                    n_ = w1.tile([P, C], F16, tag="n")
                    v.tensor_tensor(n_, cr[:, 0, :], cr[:, 1, :], op=Alu.is_gt)

                    # ---------------- selects (as blends: a0 + m*(a1-a0)) ----------------
                    n_b2 = n_.unsqueeze(1).broadcast_to([P, 2, C])
                    dx2 = w2.tile([P, 2, C], F16, tag="dx2")
                    v.tensor_sub(dx2, tv[:, 1, 0:2, :], tv[:, 0, 0:2, :])
                    v.tensor_mul(dx2, dx2, n_b2)
                    xynr = dx2
                    v.tensor_add(xynr, tv[:, 0, 0:2, :], dx2)
                    pcr = w1.tile([P, C], F16, tag="pcr")
                    dpc = w1.tile([P, C], F16, tag="dpc")
                    v.tensor_sub(dpc, pc01[:, 1, :], pc01[:, 0, :])
                    v.tensor_mul(dpc, dpc, r)
                    v.tensor_add(pcr, pc01[:, 0, :], dpc)
                    pcnr = w1.tile([P, C], F16, tag="pcnr")
                    v.tensor_add(pcnr, pc01[:, 0, :], pc01[:, 1, :])
                    v.tensor_sub(pcnr, pcnr, pcr)

                    # ---------------- conf targets & per-box weights ----------------
                    rn = w1.tile([P, C], F16, tag="rn")
                    v.tensor_add(rn, r, n_)
                    rn1 = w1.tile([P, C], F16, tag="rn1")
                    v.tensor_scalar(rn1, rn, -1.0, 1.0, op0=Alu.mult, op1=Alu.add)
                    o5 = w1.tile([P, C], F16, tag="o5")
                    v.tensor_scalar_mul(o5, obj, SQRT5)
                    osh = w1.tile([P, C], F16, tag="osh")
                    v.tensor_scalar_mul(osh, obj, SQRTH)
                    nsh = w1.tile([P, C], F16, tag="nsh")
                    v.tensor_scalar(nsh, osh, -1.0, SQRTH, op0=Alu.mult, op1=Alu.add)
                    # per-box sqrt(5)*obj*[b == resp] masks: m1 = o5*r, m0 = o5 - m1
                    mb = w2.tile([P, 2, C], F16, tag="mb")
                    v.tensor_mul(mb[:, 1, :], o5, r)
                    v.tensor_sub(mb[:, 0, :], o5, mb[:, 1, :])
                    mb_q = mb.unsqueeze(2).broadcast_to([P, 2, 2, C])

                    # ---------------- fxy = u - round(u - 0.5005) ----------------
                    # frac with integer u -> 1 (ceil semantics); the 5e-4 bias
                    # keeps u=k exact-integer cells on the fxy=1 branch.
                    u_t = w2.tile([P, 2, C], F16, tag="ut")
                    v.tensor_scalar_mul(u_t, xynr, 7.0)
                    # (u - 0.5005) + 1032 lands in [1024, 2048) where fp16 has
                    # grain exactly 1.0 -> the fp16 store rounds to an integer.
                    r_t = w2.tile([P, 2, C], F16, tag="rt")
                    v.tensor_scalar(r_t, u_t, 0.5005, 1032.0, op0=Alu.subtract, op1=Alu.add)
                    v.tensor_scalar(r_t, r_t, 1032.0, None, op0=Alu.subtract)
                    fxy = w2.tile([P, 2, C], F16, tag="fr")
                    v.tensor_sub(fxy, u_t, r_t)

                    # ---------------- residual planes (both-box for xy/wh) ----------------
                    Rv = R[:, 0:8, :].rearrange("p (b q) c -> p b q c", b=2)
                    fxy_b = fxy.unsqueeze(1).broadcast_to([P, 2, 2, C])
                    v.tensor_sub(t_dd, pxy, fxy_b)                     # dxyb
                    v.tensor_mul(Rv[:, :, 0:2, :], t_dd, mb_q)

                    s.activation(t_ss, pwh, Act.Sqrt, bias=eps_b)      # sp4
                    s.activation(t_mm, twh, Act.Sqrt, bias=eps_b)      # st4
                    v.tensor_sub(t_dd, t_ss, t_mm)                     # dwh4
                    v.tensor_mul(Rv[:, :, 2:4, :], t_dd, mb_q)

                    dc = w1.tile([P, C], F16, tag="dc")
                    v.tensor_sub(dc, pcr, rn)
                    v.tensor_mul(R[:, 8, :], dc, obj)
                    dna = w1.tile([P, C], F16, tag="dna")
                    v.tensor_sub(dna, pcnr, rn1)
                    v.tensor_mul(R[:, 9, :], dna, osh)
                    nsh_b2 = nsh.unsqueeze(1).broadcast_to([P, 2, C])
                    v.tensor_mul(R[:, 10:12, :], pc01, nsh_b2)


                    if variant == "debug" and rep == 0 and gi == 0:
                        nc.sync.dma_start(
                            out=dbg_R[:], in_=R.rearrange("p q c -> p (q c)")
                        )
                        nc.sync.dma_start(out=dbg_r[:][:, 0:C], in_=r)
                        nc.sync.dma_start(out=dbg_r[:][:, C : 2 * C], in_=n_)
                        nc.sync.dma_start(
                            out=dbg_fxy[:], in_=fxy.rearrange("p q c -> p (q c)")
                        )

                    # ---------------- square + reduce (box+conf planes) ----------------
                    s.activation(
                        R[:, 8:12, :],
                        R[:, 8:12, :],
                        Act.Square,
                        accum_out=acc32[:, 6 * col + 4 : 6 * col + 5],
                    )
                    s.activation(
                        R[:, 0:8, :],
                        R[:, 0:8, :],
                        Act.Square,
                        accum_out=acc32[:, 6 * col + 5 : 6 * col + 6],
                    )


            # ---------------- store partial columns; host sums ----------------
            nc.sync.dma_start(out=out_d[:], in_=acc32[:])

    nc.compile()
    return nc


def _prep_core(all3: np.ndarray, core: int):
    """all3: fp16 [B, 49, 60] = concat(pre, tgt). Returns (pre, tgt, cls)."""
    rows = slice(core * BPC, (core + 1) * BPC)

    def planes(chs):
        k = len(chs)
        a = all3[rows][:, :, chs]
        a = np.ascontiguousarray(a.transpose(2, 0, 1))
        a = a.reshape(k, P, G, C).transpose(2, 1, 0, 3)
        return np.ascontiguousarray(a).reshape(G, P, k * C)

    return (
        planes(PRE_CH),
        planes([30 + ch for ch in TGT_CH]),
        planes(CLS_CH),
    )


_NC_CACHE = None


def kernel(pre: np.ndarray, target: np.ndarray) -> np.ndarray:
    global _NC_CACHE
    if _NC_CACHE is None:
        _NC_CACHE = _build()
    nc = _NC_CACHE

    pre3 = np.asarray(pre, dtype=np.float32).reshape(B, 49, 30)
    tgt3 = np.asarray(target, dtype=np.float32).reshape(B, 49, 30)
    all3 = np.concatenate([pre3, tgt3], axis=2).astype(np.float16)
    in_maps = []
    for i in range(NCORES):
        pp, tp, kp = _prep_core(all3, i)
        in_maps.append({"pre": pp, "tgt": tp, "cls": kp})
    res = run_bass_kernel_spmd(nc, in_maps, core_ids=list(range(NCORES)))
    total = 0.0
    for r in res.results:
        total += float(np.sum(r["out"].astype(np.float64)))
    return np.float32(total / B)



# revision 2
# speedup vs baseline: 1.0517x; 1.0517x over previous
"""Trainium2 Bass kernel for the YOLO-style grid loss (nn_Loss_12326556139840).

Strategy: data parallel over 8 NeuronCores with host-side obj/noobj cell
compaction (layout-only prep; all value arithmetic stays on device).

Observation: every loss term except the no-object confidence term is
masked by cell_obj; no-object cells (about half, since obj is a coin
flip) contribute ONLY 0.5*(pc0^2 + pc1^2).  So the host gathers the obj
cells into a dense plane-major fp16 layout (18 box planes + 40 class
planes per cell) and ships just the two predicted-conf planes for the
noobj cells.  This halves HBM traffic and removes every obj-mask
multiply from the device program (on the obj partition obj == 1).

Padding cells are synthesized to contribute exactly zero to all terms:
pre = [x0=1,y0=1,x1=0,y1=0, wh=0, c0=0,c1=1], tgt = 0.  (Both IoUs tie
-> r=n=0 -> resp=0, nonresp=0; fxy(0)=1 so the xy residual of box 0 is
1-1=0; conf targets are rn=0 for the resp slot and 1-rn=1 for the
non-resp slot, matching c0=0, c1=1.)

Device-side math per obj cell (branchless, all fp16 unit-stride):
  - iw = relu(min(pw+tw-2|px-tx|, 2*min(pw,tw)))  (2x-scaled overlap)
  - responsible box via cross-multiplied IoU compare (i1*A0 vs i0*A1;
    A = sum of areas; the i0*i1 union terms cancel; +4e-4 keeps the
    reference's eps tie-break)
  - fxy = frac(7*xy_nr) with frac<=0 -> 1, via fp16 +1032 rounding
  - residual planes masked by [b==resp] only; loss weights are folded
    into the ACT Square scale (sqrt5 for xy/wh, sqrt.5 for no-conf).

Engine split: DVE does the plane algebra (2x/4x perf modes), GPSIMD the
per-cell scalar chains (xy_nr blend, fxy, conf targets), ACT abs/sqrt
and all Square+accum reductions.  Output: [128, 7] fp32 accumulator
columns per core; the host sums and divides by B.
"""

import numpy as np

import concourse.bacc as bacc
import concourse.tile as tile
from concourse import mybir
from concourse.bass_utils import run_bass_kernel_spmd

F32 = mybir.dt.float32
F16 = mybir.dt.float16
Alu = mybir.AluOpType
Act = mybir.ActivationFunctionType

B = 16384
NCORES = 8
NCELL = B * 49               # 802816 cells total
P = 128
G = 2                        # pipeline groups
C = 204                      # obj cells per partition per group
CAP = G * P * C              # 52224 obj-cell capacity per core
CN = G * C                   # noobj cells per partition (single group)
CAPN = P * CN                # 52224 noobj-cell capacity per core

EPS = 1e-7
SQRT5 = float(np.sqrt(5.0))
SQRTH = float(np.sqrt(0.5))

# box-plane channel picks from concat(pre, tgt) [.., 60]:
#   pxy4 (x0,y0,x1,y1) | pwh4 (w0,h0,w1,h1) | pc2 | txy4 | twh4
BOX_CH = [0, 1, 5, 6, 2, 3, 7, 8, 4, 9,
          30, 31, 35, 36, 32, 33, 37, 38]
CLS_CH = list(range(10, 30)) + list(range(40, 60))
# pad cell: zero contribution to every loss term (see module docstring)
PAD_ROW = np.array(
    [1, 1, 0, 0, 0, 0, 0, 0, 0, 1, 0, 0, 0, 0, 0, 0, 0, 0],
    dtype=np.float16,
)


def _build():
    nc = bacc.Bacc()
    box_d = nc.declare_dram_parameter("box", [G, P, 18 * C], F16, isOutput=False)
    cls_d = nc.declare_dram_parameter("cls", [G, P, 40 * C], F16, isOutput=False)
    nob_d = nc.declare_dram_parameter("nob", [P, 2 * CN], F16, isOutput=False)
    out_d = nc.declare_dram_parameter("out", [P, 7], F32, isOutput=True)

    with tile.TileContext(nc) as tc:
        with (
            tc.tile_pool(name="bx", bufs=2) as bxp,
            tc.tile_pool(name="kl", bufs=2) as klp,
            tc.tile_pool(name="rr", bufs=2) as rrp,
            tc.tile_pool(name="w4", bufs=2) as w4,
            tc.tile_pool(name="w2", bufs=2) as w2,
            tc.tile_pool(name="w1", bufs=2) as w1,
            tc.tile_pool(name="one", bufs=1) as one,
        ):
            v = nc.vector
            s = nc.scalar
            g_ = nc.gpsimd

            acc = one.tile([P, 7], F32, tag="acc")
            v.memset(acc, 0.0)
            eps_b = one.tile([P, 1], F32, tag="eps")
            v.memset(eps_b, EPS)

            # ---- issue all input DMAs up front (bufs=2 keeps both live) ----
            tiles = []
            for gi in range(G):
                bx = bxp.tile([P, 18, C], F16, tag="bx")
                kl = klp.tile([P, 40, C], F16, tag="kl")
                nc.sync.dma_start(
                    out=bx, in_=box_d[gi].rearrange("p (q c) -> p q c", c=C)
                )
                nc.sync.dma_start(
                    out=kl, in_=cls_d[gi].rearrange("p (q c) -> p q c", c=C)
                )
                tiles.append((bx, kl))
            nob = one.tile([P, 2, CN], F16, tag="nob")
            nc.sync.dma_start(
                out=nob, in_=nob_d[:].rearrange("p (q c) -> p q c", c=CN)
            )

            for gi in range(G):
                bx, kl = tiles[gi]
                pxy4 = bx[:, 0:4, :]
                pwh4 = bx[:, 4:8, :]
                pc2 = bx[:, 8:10, :]
                txy4 = bx[:, 10:14, :]
                twh4 = bx[:, 14:18, :]
                R = rrp.tile([P, 30, C], F16, tag="R")

                # ---------------- IoU -> r, n (DVE + ACT abs) ----------------
                t_d = w4.tile([P, 4, C], F16, tag="d")
                t_s = w4.tile([P, 4, C], F16, tag="s")
                t_m = w4.tile([P, 4, C], F16, tag="m")
                v.tensor_sub(t_d, pxy4, txy4)
                s.activation(t_d, t_d, Act.Abs, scale=2.0)      # a4 = 2|d|
                v.tensor_add(t_s, pwh4, twh4)
                v.tensor_tensor(t_m, pwh4, twh4, op=Alu.min)
                v.tensor_scalar_mul(t_m, t_m, 2.0)              # mm4
                v.tensor_sub(t_s, t_s, t_d)                     # e4
                v.tensor_tensor(t_s, t_m, t_s, op=Alu.min)      # iw4
                v.tensor_single_scalar(t_s, t_s, 0.0, op=Alu.max)
                t_i = w2.tile([P, 2, C], F16, tag="i")
                v.tensor_mul(t_i, t_s[:, 0::2, :], t_s[:, 1::2, :])   # i2
                t_ap = w2.tile([P, 2, C], F16, tag="ap")
                t_at = w2.tile([P, 2, C], F16, tag="at")
                v.tensor_mul(t_ap, pwh4[:, 0::2, :], pwh4[:, 1::2, :])
                v.tensor_mul(t_at, twh4[:, 0::2, :], twh4[:, 1::2, :])
                v.tensor_add(t_ap, t_ap, t_at)                  # A2
                v.tensor_scalar_add(t_i, t_i, 4e-4)
                t_cr = w2.tile([P, 2, C], F16, tag="cr")
                v.tensor_mul(t_cr, t_i, t_ap[:, ::-1, :])
                r_ = w1.tile([P, C], F16, tag="r")
                n_ = w1.tile([P, C], F16, tag="n")
                v.tensor_tensor(r_, t_cr[:, 1, :], t_cr[:, 0, :], op=Alu.is_gt)
                v.tensor_tensor(n_, t_cr[:, 0, :], t_cr[:, 1, :], op=Alu.is_gt)

                # ------------- per-cell scalar chains (GPSIMD) -------------
                # xy of the non-responsible target box, then fxy
                t_d2 = w2.tile([P, 2, C], F16, tag="d2")
                n_b2 = n_.unsqueeze(1).broadcast_to([P, 2, C])
                g_.tensor_sub(t_d2, txy4[:, 2:4, :], txy4[:, 0:2, :])
                g_.tensor_mul(t_d2, t_d2, n_b2)
                g_.tensor_add(t_d2, txy4[:, 0:2, :], t_d2)      # xy_nr
                t_u = w2.tile([P, 2, C], F16, tag="u")
                t_rt = w2.tile([P, 2, C], F16, tag="rt")
                g_.tensor_scalar_mul(t_u, t_d2, 7.0)
                # (u - 0.5005) + 1032 lands in [1024, 2048) where fp16 grain
                # is exactly 1.0 -> the fp16 store rounds to an integer.
                g_.tensor_scalar(t_rt, t_u, 0.5005, 1032.0,
                                 op0=Alu.subtract, op1=Alu.add)
                g_.tensor_scalar(t_rt, t_rt, 1032.0, None, op0=Alu.subtract)
                g_.tensor_sub(t_u, t_u, t_rt)                   # fxy
                # conf targets: resp slot rn = r+n, non-resp slot 1-rn
                dpc = w1.tile([P, C], F16, tag="dpc")
                pcr = w1.tile([P, C], F16, tag="pcr")
                pcs = w1.tile([P, C], F16, tag="pcs")
                rn = w1.tile([P, C], F16, tag="rn")
                g_.tensor_sub(dpc, pc2[:, 1, :], pc2[:, 0, :])
                g_.tensor_mul(dpc, dpc, r_)
                g_.tensor_add(pcr, pc2[:, 0, :], dpc)           # resp conf
                g_.tensor_add(pcs, pc2[:, 0, :], pc2[:, 1, :])
                g_.tensor_add(rn, r_, n_)
                g_.tensor_sub(R[:, 8, :], pcr, rn)              # dc
                g_.tensor_sub(pcs, pcs, pcr)                    # non-resp conf
                g_.tensor_add(pcs, pcs, rn)
                g_.tensor_scalar(R[:, 9, :], pcs, 1.0, SQRTH,
                                 op0=Alu.subtract, op1=Alu.mult)  # sqrt(.5)*dna

                # ------------- class diffs + box residuals (DVE) -------------
                v.tensor_sub(R[:, 10:30, :], kl[:, 0:20, :], kl[:, 20:40, :])
                t_sp = w4.tile([P, 4, C], F16, tag="sp")
                t_st = w4.tile([P, 4, C], F16, tag="st")
                s.activation(t_sp, pwh4, Act.Sqrt, bias=eps_b)
                s.activation(t_st, twh4, Act.Sqrt, bias=eps_b)
                v.tensor_sub(R[:, 2:4, :], t_sp[:, 0:2, :], t_st[:, 0:2, :])
                v.tensor_sub(R[:, 6:8, :], t_sp[:, 2:4, :], t_st[:, 2:4, :])
                m0 = w1.tile([P, C], F16, tag="m0")
                v.tensor_scalar(m0, r_, -1.0, 1.0, op0=Alu.mult, op1=Alu.add)
                v.tensor_sub(R[:, 0:2, :], pxy4[:, 0:2, :], t_u)
                v.tensor_sub(R[:, 4:6, :], pxy4[:, 2:4, :], t_u)
                m0_b4 = m0.unsqueeze(1).broadcast_to([P, 4, C])
                r_b4 = r_.unsqueeze(1).broadcast_to([P, 4, C])
                v.tensor_mul(R[:, 0:4, :], R[:, 0:4, :], m0_b4)
                v.tensor_mul(R[:, 4:8, :], R[:, 4:8, :], r_b4)

                # ---------------- square + accumulate (ACT) ----------------
                col = 3 * gi
                s.activation(R[:, 10:30, :], R[:, 10:30, :], Act.Square,
                             accum_out=acc[:, col + 1 : col + 2])
                s.activation(R[:, 0:8, :], R[:, 0:8, :], Act.Square,
                             scale=SQRT5,
                             accum_out=acc[:, col : col + 1])
                s.activation(R[:, 8:10, :], R[:, 8:10, :], Act.Square,
                             accum_out=acc[:, col + 2 : col + 3])

            # no-object cells: 0.5 * sum(pc^2)
            s.activation(nob, nob, Act.Square, scale=SQRTH,
                         accum_out=acc[:, 6:7])

            nc.sync.dma_start(out=out_d[:], in_=acc[:])

    nc.compile()
    return nc


def _prep_core(all16: np.ndarray, obj_idx, non_idx, core: int):
    """all16: fp16 [NCELL, 60] = concat(pre, tgt) flattened per cell."""
    qo = (len(obj_idx) + NCORES - 1) // NCORES
    qn = (len(non_idx) + NCORES - 1) // NCORES
    assert qo <= CAP and qn <= CAPN, (qo, qn)
    oi = obj_idx[core * qo : (core + 1) * qo]
    ni = non_idx[core * qn : (core + 1) * qn]

    gob = all16[oi]
    box = np.empty((CAP, 18), dtype=np.float16)
    box[:] = PAD_ROW
    box[: len(oi)] = gob[:, BOX_CH]
    cls = np.zeros((CAP, 40), dtype=np.float16)
    cls[: len(oi)] = gob[:, CLS_CH]
    box = np.ascontiguousarray(
        box.reshape(G, P, C, 18).transpose(0, 1, 3, 2)
    ).reshape(G, P, 18 * C)
    cls = np.ascontiguousarray(
        cls.reshape(G, P, C, 40).transpose(0, 1, 3, 2)
    ).reshape(G, P, 40 * C)

    nob = np.zeros((CAPN, 2), dtype=np.float16)
    nob[: len(ni)] = all16[ni][:, [4, 9]]
    nob = np.ascontiguousarray(
        nob.reshape(P, CN, 2).transpose(0, 2, 1)
    ).reshape(P, 2 * CN)
    return {"box": box, "cls": cls, "nob": nob}


_NC_CACHE = None


def kernel(pre: np.ndarray, target: np.ndarray) -> np.ndarray:
    global _NC_CACHE
    if _NC_CACHE is None:
        _NC_CACHE = _build()
    nc = _NC_CACHE

    pre3 = np.asarray(pre, dtype=np.float32).reshape(NCELL, 30)
    tgt3 = np.asarray(target, dtype=np.float32).reshape(NCELL, 30)
    objmask = tgt3[:, 4] > 0
    all16 = np.concatenate(
        [pre3.astype(np.float16), tgt3.astype(np.float16)], axis=1
    )
    obj_idx = np.flatnonzero(objmask)
    non_idx = np.flatnonzero(~objmask)

    in_maps = [
        _prep_core(all16, obj_idx, non_idx, i) for i in range(NCORES)
    ]
    res = run_bass_kernel_spmd(nc, in_maps, core_ids=list(range(NCORES)))
    total = 0.0
    for r in res.results:
        total += float(np.sum(r["out"].astype(np.float64)))
    return np.float32(total / B)


# revision 6
# speedup vs baseline: 1.9146x; 1.8204x over previous
"""Trainium2 Bass kernel for the YOLO-style grid loss (nn_Loss_12326556139840).

Strategy: data parallel over 8 NeuronCores with host-side obj/noobj cell
compaction (layout-only prep; all value arithmetic stays on device).

Observation: every loss term except the no-object confidence term is
masked by cell_obj; no-object cells (about half, since obj is a coin
flip) contribute ONLY 0.5*(pc0^2 + pc1^2).  So the host gathers the obj
cells into a dense plane-major fp16 layout (18 box planes + 40 class
planes per cell) and ships just the two predicted-conf planes for the
noobj cells.  This halves HBM traffic and removes every obj-mask
multiply from the device program (on the obj partition obj == 1).

Padding cells are synthesized to contribute exactly zero to all terms:
pre = [x0=1,y0=1,x1=0,y1=0, wh=0, c0=0,c1=1], tgt = 0.  (Both IoUs tie
-> r=n=0 -> resp=0, nonresp=0; fxy(0)=1 so the xy residual of box 0 is
1-1=0; conf targets are rn=0 for the resp slot and 1-rn=1 for the
non-resp slot, matching c0=0, c1=1.)

Device-side math per obj cell (branchless, all fp16 unit-stride):
  - iw = relu(min(pw+tw-2|px-tx|, 2*min(pw,tw)))  (2x-scaled overlap)
  - responsible box via cross-multiplied IoU compare (i1*A0 vs i0*A1;
    A = sum of areas; the i0*i1 union terms cancel; +4e-4 keeps the
    reference's eps tie-break)
  - fxy = frac(7*xy_nr) with frac<=0 -> 1, via fp16 +1032 rounding
  - residual planes masked by [b==resp] only; loss weights are folded
    into the ACT Square scale (sqrt5 for xy/wh, sqrt.5 for no-conf).

Engine split: DVE does the plane algebra (2x/4x perf modes), GPSIMD the
per-cell scalar chains (xy_nr blend, fxy, conf targets), ACT abs/sqrt
and all Square+accum reductions.  Output: [128, 7] fp32 accumulator
columns per core; the host sums and divides by B.
"""

import numpy as np

import concourse.bacc as bacc
import concourse.tile as tile
from concourse import mybir
from concourse.bass_utils import run_bass_kernel_spmd

F32 = mybir.dt.float32
F16 = mybir.dt.float16
Alu = mybir.AluOpType
Act = mybir.ActivationFunctionType

B = 16384
NCORES = 8
NCELL = B * 49               # 802816 cells total
P = 128
G = 2                        # pipeline groups
C = 204                      # obj cells per partition per group
CAP = G * P * C              # 52224 obj-cell capacity per core
CN = G * C                   # noobj cells per partition (single group)
CAPN = P * CN                # 52224 noobj-cell capacity per core

EPS = 1e-7
SQRT5 = float(np.sqrt(5.0))
SQRTH = float(np.sqrt(0.5))

# box-plane channel picks from concat(pre, tgt) [.., 60]:
#   pxy4 (x0,y0,x1,y1) | txy4 | pwh4 (w0,h0,w1,h1) | twh4 | pc2
BOX_CH = [0, 1, 5, 6, 30, 31, 35, 36,
          2, 3, 7, 8, 32, 33, 37, 38, 4, 9]
CLS_CH = list(range(10, 30)) + list(range(40, 60))
# pad cell: zero contribution to every loss term (see module docstring)
PAD_ROW = np.array(
    [1, 1, 0, 0, 0, 0, 0, 0, 0, 0, 0, 0, 0, 0, 0, 0, 0, 1],
    dtype=np.float16,
)


def _build():
    nc = bacc.Bacc()
    box_d = nc.declare_dram_parameter("box", [G, P, 18 * C], F16, isOutput=False)
    cls_d = nc.declare_dram_parameter("cls", [G, P, 40 * C], F16, isOutput=False)
    nob_d = nc.declare_dram_parameter("nob", [P, 2 * CN], F16, isOutput=False)
    out_d = nc.declare_dram_parameter("out", [P, 7], F32, isOutput=True)

    with tile.TileContext(nc) as tc:
        with (
            tc.tile_pool(name="bx", bufs=2) as bxp,
            tc.tile_pool(name="kl", bufs=2) as klp,
            tc.tile_pool(name="rr", bufs=2) as rrp,
            tc.tile_pool(name="w4", bufs=2) as w4,
            tc.tile_pool(name="w2", bufs=2) as w2,
            tc.tile_pool(name="w1", bufs=2) as w1,
            tc.tile_pool(name="one", bufs=1) as one,
        ):
            v = nc.vector
            s = nc.scalar
            g_ = nc.gpsimd

            acc = one.tile([P, 7], F32, tag="acc")
            v.memset(acc, 0.0)
            eps_b = one.tile([P, 1], F32, tag="eps")
            v.memset(eps_b, EPS)

            # ---- issue all input DMAs up front (bufs=2 keeps both live) ----
            tiles = []
            for gi in range(G):
                bx = bxp.tile([P, 18, C], F16, tag="bx")
                kl = klp.tile([P, 40, C], F16, tag="kl")
                box_v = box_d[gi].rearrange("p (q c) -> p q c", c=C)
                nc.sync.dma_start(out=bx[:, 0:8, :], in_=box_v[:, 0:8, :])
                nc.sync.dma_start(out=bx[:, 8:18, :], in_=box_v[:, 8:18, :])
                nc.sync.dma_start(
                    out=kl, in_=cls_d[gi].rearrange("p (q c) -> p q c", c=C)
                )
                tiles.append((bx, kl))
            nob = one.tile([P, 2, CN], F16, tag="nob")
            nc.sync.dma_start(
                out=nob, in_=nob_d[:].rearrange("p (q c) -> p q c", c=CN)
            )

            for gi in range(G):
                bx, kl = tiles[gi]
                pxy4 = bx[:, 0:4, :]
                txy4 = bx[:, 4:8, :]
                pwh4 = bx[:, 8:12, :]
                twh4 = bx[:, 12:16, :]
                pc2 = bx[:, 16:18, :]
                R = rrp.tile([P, 30, C], F16, tag="R")

                # ---------------- IoU -> r, n (DVE + ACT + GPSIMD) ----------------
                t_d = w4.tile([P, 4, C], F16, tag="d")
                t_s = w4.tile([P, 4, C], F16, tag="s")
                t_m = w4.tile([P, 4, C], F16, tag="m")
                v.tensor_sub(t_d, pxy4, txy4)
                s.activation(t_d, t_d, Act.Abs, scale=2.0)      # a4 = 2|d|
                v.tensor_add(t_s, pwh4, twh4)                   # s4
                v.tensor_tensor(t_m, pwh4, twh4, op=Alu.min)    # m4
                v.tensor_scalar_mul(t_m, t_m, 2.0)              # mm4
                v.tensor_sub(t_s, t_s, t_d)                     # e4
                v.tensor_tensor(t_s, t_m, t_s, op=Alu.min)      # iw4
                v.tensor_single_scalar(t_s, t_s, 0.0, op=Alu.max)
                t_i = w2.tile([P, 2, C], F16, tag="i")
                v.tensor_mul(t_i, t_s[:, 0::2, :], t_s[:, 1::2, :])   # i2
                t_ap = w2.tile([P, 2, C], F16, tag="ap")
                t_at = w2.tile([P, 2, C], F16, tag="at")
                g_.tensor_mul(t_ap, pwh4[:, 0::2, :], pwh4[:, 1::2, :])
                g_.tensor_mul(t_at, twh4[:, 0::2, :], twh4[:, 1::2, :])
                g_.tensor_add(t_ap, t_ap, t_at)                 # A2
                v.tensor_scalar_add(t_i, t_i, 4e-4)
                t_cr = w2.tile([P, 2, C], F16, tag="cr")
                v.tensor_mul(t_cr, t_i, t_ap[:, ::-1, :])
                r_ = w1.tile([P, C], F16, tag="r")
                n_ = w1.tile([P, C], F16, tag="n")
                v.tensor_tensor(r_, t_cr[:, 1, :], t_cr[:, 0, :], op=Alu.is_gt)
                v.tensor_tensor(n_, t_cr[:, 0, :], t_cr[:, 1, :], op=Alu.is_gt)

                # ------------- class diffs early (feeds the big square) -------------
                v.tensor_sub(R[:, 10:30, :], kl[:, 0:20, :], kl[:, 20:40, :])

                # ------- conf targets (GPSIMD): resp slot r+n, non-resp 1-(r+n) -------
                dpc = w1.tile([P, C], F16, tag="dpc")
                pcr = w1.tile([P, C], F16, tag="pcr")
                pcs = w1.tile([P, C], F16, tag="pcs")
                rn = w1.tile([P, C], F16, tag="rn")
                g_.tensor_sub(dpc, pc2[:, 1, :], pc2[:, 0, :])
                g_.tensor_mul(dpc, dpc, r_)
                g_.tensor_add(pcr, pc2[:, 0, :], dpc)           # resp conf
                g_.tensor_add(pcs, pc2[:, 0, :], pc2[:, 1, :])
                g_.tensor_add(rn, r_, n_)
                g_.tensor_sub(R[:, 8, :], pcr, rn)              # dc
                g_.tensor_sub(pcs, pcs, pcr)                    # non-resp conf
                g_.tensor_add(R[:, 9, :], pcs, rn)              # dna + 1

                # ------------- xy_nr blend + fxy (DVE) -------------
                t_d2 = w2.tile([P, 2, C], F16, tag="d2")
                n_b2 = n_.unsqueeze(1).broadcast_to([P, 2, C])
                v.tensor_sub(t_d2, txy4[:, 2:4, :], txy4[:, 0:2, :])
                v.tensor_mul(t_d2, t_d2, n_b2)
                v.tensor_add(t_d2, txy4[:, 0:2, :], t_d2)       # xy_nr
                t_u = w2.tile([P, 2, C], F16, tag="u")
                t_rt = w2.tile([P, 2, C], F16, tag="rt")
                v.tensor_scalar_mul(t_u, t_d2, 7.0)
                # (u - 0.5005) + 1032 lands in [1024, 2048) where fp16 grain
                # is exactly 1.0 -> the fp16 store rounds to an integer.
                v.tensor_scalar(t_rt, t_u, 0.5005, 1032.0,
                                op0=Alu.subtract, op1=Alu.add)
                v.tensor_scalar(t_rt, t_rt, 1032.0, None, op0=Alu.subtract)
                v.tensor_sub(t_u, t_u, t_rt)                    # fxy

                # ------------- box residuals + masks (DVE, ACT sqrt) -------------
                t_sp = w4.tile([P, 4, C], F16, tag="sp")
                t_st = w4.tile([P, 4, C], F16, tag="st")
                s.activation(t_sp, pwh4, Act.Sqrt, bias=eps_b)
                s.activation(t_st, twh4, Act.Sqrt, bias=eps_b)
                # R[0:8] = [xy0, wh0, xy1, wh1] residuals via [b, q] views
                Rb = R[:, 0:8, :].rearrange("p (b q) c -> p b q c", b=2)
                pxy_b = pxy4.rearrange("p (b q) c -> p b q c", b=2)
                fxy_b = t_u.unsqueeze(1).broadcast_to([P, 2, 2, C])
                v.tensor_sub(Rb[:, :, 0:2, :], pxy_b, fxy_b)
                sp_b = t_sp.rearrange("p (b q) c -> p b q c", b=2)
                st_b = t_st.rearrange("p (b q) c -> p b q c", b=2)
                v.tensor_sub(Rb[:, :, 2:4, :], sp_b, st_b)
                m0 = w1.tile([P, C], F16, tag="m0")
                v.tensor_scalar(m0, r_, -1.0, 1.0, op0=Alu.mult, op1=Alu.add)
                m0_b4 = m0.unsqueeze(1).broadcast_to([P, 4, C])
                r_b4 = r_.unsqueeze(1).broadcast_to([P, 4, C])
                v.tensor_mul(R[:, 0:4, :], R[:, 0:4, :], m0_b4)
                v.tensor_mul(R[:, 4:8, :], R[:, 4:8, :], r_b4)
                # sqrt(.5)*dna  (GPSIMD wrote dna+1 into R[9])
                v.tensor_scalar(R[:, 9, :], R[:, 9, :], 1.0, SQRTH,
                                op0=Alu.subtract, op1=Alu.mult)

                # ---------------- square + accumulate (ACT) ----------------
                col = 3 * gi
                s.activation(R[:, 10:30, :], R[:, 10:30, :], Act.Square,
                             accum_out=acc[:, col + 1 : col + 2])
                s.activation(R[:, 0:8, :], R[:, 0:8, :], Act.Square,
                             scale=SQRT5,
                             accum_out=acc[:, col : col + 1])
                s.activation(R[:, 8:10, :], R[:, 8:10, :], Act.Square,
                             accum_out=acc[:, col + 2 : col + 3])

            # no-object cells: 0.5 * sum(pc^2)
            s.activation(nob, nob, Act.Square, scale=SQRTH,
                         accum_out=acc[:, 6:7])

            nc.sync.dma_start(out=out_d[:], in_=acc[:])

    nc.compile()
    return nc


def _prep_core(all16: np.ndarray, obj_idx, non_idx, core: int):
    """all16: fp16 [NCELL, 60] = concat(pre, tgt) flattened per cell."""
    qo = (len(obj_idx) + NCORES - 1) // NCORES
    qn = (len(non_idx) + NCORES - 1) // NCORES
    assert qo <= CAP and qn <= CAPN, (qo, qn)
    oi = obj_idx[core * qo : (core + 1) * qo]
    ni = non_idx[core * qn : (core + 1) * qn]

    gob = all16[oi]
    box = np.empty((CAP, 18), dtype=np.float16)
    box[:] = PAD_ROW
    box[: len(oi)] = gob[:, BOX_CH]
    cls = np.zeros((CAP, 40), dtype=np.float16)
    cls[: len(oi)] = gob[:, CLS_CH]
    box = np.ascontiguousarray(
        box.reshape(G, P, C, 18).transpose(0, 1, 3, 2)
    ).reshape(G, P, 18 * C)
    cls = np.ascontiguousarray(
        cls.reshape(G, P, C, 40).transpose(0, 1, 3, 2)
    ).reshape(G, P, 40 * C)

    nob = np.zeros((CAPN, 2), dtype=np.float16)
    nob[: len(ni)] = all16[ni][:, [4, 9]]
    nob = np.ascontiguousarray(
        nob.reshape(P, CN, 2).transpose(0, 2, 1)
    ).reshape(P, 2 * CN)
    return {"box": box, "cls": cls, "nob": nob}


_NC_CACHE = None


def kernel(pre: np.ndarray, target: np.ndarray) -> np.ndarray:
    global _NC_CACHE
    if _NC_CACHE is None:
        _NC_CACHE = _build()
    nc = _NC_CACHE

    pre3 = np.asarray(pre, dtype=np.float32).reshape(NCELL, 30)
    tgt3 = np.asarray(target, dtype=np.float32).reshape(NCELL, 30)
    objmask = tgt3[:, 4] > 0
    all16 = np.concatenate(
        [pre3.astype(np.float16), tgt3.astype(np.float16)], axis=1
    )
    obj_idx = np.flatnonzero(objmask)
    non_idx = np.flatnonzero(~objmask)

    in_maps = [
        _prep_core(all16, obj_idx, non_idx, i) for i in range(NCORES)
    ]
    res = run_bass_kernel_spmd(nc, in_maps, core_ids=list(range(NCORES)))
    total = 0.0
    for r in res.results:
        total += float(np.sum(r["out"].astype(np.float64)))
    return np.float32(total / B)


# revision 10
# speedup vs baseline: 2.1369x; 1.1161x over previous
"""Trainium2 Bass kernel for the YOLO-style grid loss (nn_Loss_12326556139840).

Strategy: data parallel over 8 NeuronCores with host-side obj/noobj cell
compaction (layout-only prep; all value arithmetic stays on device).

Observation: every loss term except the no-object confidence term is
masked by cell_obj; no-object cells (about half, since obj is a coin
flip) contribute ONLY 0.5*(pc0^2 + pc1^2).  So the host gathers the obj
cells into a dense plane-major fp16 layout (18 box planes + 40 class
planes per cell) and ships just the two predicted-conf planes for the
noobj cells.  This halves HBM traffic and removes every obj-mask
multiply from the device program (on the obj partition obj == 1).

Padding cells are synthesized to contribute exactly zero to all terms:
pre = [x0=1,y0=1,x1=0,y1=0, wh=0, c0=0,c1=1], tgt = 0.  (Both IoUs tie
-> r=n=0 -> resp=0, nonresp=0; fxy(0)=1 so the xy residual of box 0 is
1-1=0; conf targets are rn=0 for the resp slot and 1-rn=1 for the
non-resp slot, matching c0=0, c1=1.)

Device-side math per obj cell (branchless, all fp16 unit-stride):
  - iw = relu(min(pw+tw-2|px-tx|, 2*min(pw,tw)))  (2x-scaled overlap)
  - responsible box via cross-multiplied IoU compare (i1*A0 vs i0*A1;
    A = sum of areas; the i0*i1 union terms cancel; +4e-4 keeps the
    reference's eps tie-break)
  - fxy = frac(7*xy_nr) with frac<=0 -> 1, via fp16 +1032 rounding
  - residual planes masked by [b==resp] only; loss weights are folded
    into the ACT Square scale (sqrt5 for xy/wh, sqrt.5 for no-conf).

Engine split: DVE does the plane algebra (2x/4x perf modes), GPSIMD the
per-cell scalar chains (xy_nr blend, fxy, conf targets), ACT abs/sqrt
and all Square+accum reductions.  Output: [128, 7] fp32 accumulator
columns per core; the host sums and divides by B.
"""

import numpy as np

import concourse.bacc as bacc
import concourse.tile as tile
from concourse import mybir
from concourse.bass_utils import run_bass_kernel_spmd

F32 = mybir.dt.float32
F16 = mybir.dt.float16
Alu = mybir.AluOpType
Act = mybir.ActivationFunctionType

B = 16384
NCORES = 8
NCELL = B * 49               # 802816 cells total
P = 128
G = 2                        # pipeline groups
C = 204                      # obj cells per partition per group
CAP = G * P * C              # 52224 obj-cell capacity per core
CN = G * C                   # noobj cells per partition (single group)
CAPN = P * CN                # 52224 noobj-cell capacity per core

EPS = 1e-7
SQRT5 = float(np.sqrt(5.0))
SQRTH = float(np.sqrt(0.5))

# box-plane channel picks from concat(pre, tgt) [.., 60]:
#   pxy4 (x0,y0,x1,y1) | txy4 | pwh4 (w0,h0,w1,h1) | twh4 | pc2
BOX_CH = [0, 1, 5, 6, 30, 31, 35, 36,
          2, 3, 7, 8, 32, 33, 37, 38, 4, 9]
CLS_CH = list(range(10, 30)) + list(range(40, 60))
# pad cell: zero contribution to every loss term (see module docstring)
PAD_ROW = np.array(
    [1, 1, 0, 0, 0, 0, 0, 0, 0, 0, 0, 0, 0, 0, 0, 0, 0, 1],
    dtype=np.float16,
)


def _build():
    nc = bacc.Bacc()
    box_d = nc.declare_dram_parameter("box", [G, P, 18 * C], F16, isOutput=False)
    cls_d = nc.declare_dram_parameter("cls", [G, P, 40 * C], F16, isOutput=False)
    nob_d = nc.declare_dram_parameter("nob", [P, 2 * CN], F16, isOutput=False)
    out_d = nc.declare_dram_parameter("out", [P, 7], F32, isOutput=True)

    with tile.TileContext(nc) as tc:
        with (
            tc.tile_pool(name="bx", bufs=2) as bxp,
            tc.tile_pool(name="kl", bufs=2) as klp,
            tc.tile_pool(name="rr", bufs=2) as rrp,
            tc.tile_pool(name="w4", bufs=2) as w4,
            tc.tile_pool(name="w2", bufs=2) as w2,
            tc.tile_pool(name="w1", bufs=2) as w1,
            tc.tile_pool(name="one", bufs=1) as one,
        ):
            v = nc.vector
            s = nc.scalar
            g_ = nc.gpsimd

            acc = one.tile([P, 7], F32, tag="acc")
            v.memset(acc, 0.0)
            eps_b = one.tile([P, 1], F32, tag="eps")
            v.memset(eps_b, EPS)

            # ---- issue all input DMAs up front (bufs=2 keeps both live) ----
            tiles = []
            for gi in range(G):
                bx = bxp.tile([P, 18, C], F16, tag="bx")
                kl = klp.tile([P, 40, C], F16, tag="kl")
                box_v = box_d[gi].rearrange("p (q c) -> p q c", c=C)
                nc.sync.dma_start(out=bx[:, 0:8, :], in_=box_v[:, 0:8, :])
                nc.sync.dma_start(out=bx[:, 8:18, :], in_=box_v[:, 8:18, :])
                nc.sync.dma_start(
                    out=kl, in_=cls_d[gi].rearrange("p (q c) -> p q c", c=C)
                )
                tiles.append((bx, kl))
            nob = one.tile([P, 2, CN], F16, tag="nob")
            nc.sync.dma_start(
                out=nob, in_=nob_d[:].rearrange("p (q c) -> p q c", c=CN)
            )

            # per-group tile handles (phases are interleaved across groups so
            # the in-order engine queues can fill one group's stalls with the
            # other group's work)
            ts_ = []
            for gi in range(G):
                d = {
                    "R": rrp.tile([P, 30, C], F16, tag="R", name="R"),
                    "d": w4.tile([P, 4, C], F16, tag="d", name="td"),
                    "s": w4.tile([P, 4, C], F16, tag="s", name="ts"),
                    "m": w4.tile([P, 4, C], F16, tag="m", name="tm"),
                    "sp": w4.tile([P, 4, C], F16, tag="sp", name="tsp"),
                    "st": w4.tile([P, 4, C], F16, tag="st", name="tst"),
                    "i": w2.tile([P, 2, C], F16, tag="i", name="ti"),
                    "ap": w2.tile([P, 2, C], F16, tag="ap", name="tap"),
                    "at": w2.tile([P, 2, C], F16, tag="at", name="tat"),
                    "cr": w2.tile([P, 2, C], F16, tag="cr", name="tcr"),
                    "d2": w2.tile([P, 2, C], F16, tag="d2", name="td2"),
                    "u": w2.tile([P, 2, C], F16, tag="u", name="tu"),
                    "rt": w2.tile([P, 2, C], F16, tag="rt", name="trt"),
                    "r": w1.tile([P, C], F16, tag="r", name="tr"),
                    "n": w1.tile([P, C], F16, tag="n", name="tn"),
                    "rn": w1.tile([P, C], F16, tag="rn", name="trn"),
                    "dpc": w1.tile([P, C], F16, tag="dpc", name="tdpc"),
                    "m0": w1.tile([P, C], F16, tag="m0", name="tm0"),
                }
                bx, kl = tiles[gi]
                d["pxy4"] = bx[:, 0:4, :]
                d["txy4"] = bx[:, 4:8, :]
                d["pwh4"] = bx[:, 8:12, :]
                d["twh4"] = bx[:, 12:16, :]
                d["pc2"] = bx[:, 16:18, :]
                d["kl"] = kl
                ts_.append(d)

            # ---- phase A: IoU -> r, n, rn (DVE only) ----
            for t in ts_:
                v.tensor_sub(t["d"], t["pxy4"], t["txy4"])
                s.activation(t["d"], t["d"], Act.Abs, scale=2.0)    # a4 = 2|d|
                v.tensor_add(t["s"], t["pwh4"], t["twh4"])          # s4
                v.tensor_tensor(t["m"], t["pwh4"], t["twh4"], op=Alu.min)
                v.tensor_scalar_mul(t["m"], t["m"], 2.0)            # mm4
                v.tensor_sub(t["s"], t["s"], t["d"])                # e4
                v.tensor_tensor(t["s"], t["m"], t["s"], op=Alu.min)  # iw4
                v.tensor_single_scalar(t["s"], t["s"], 0.0, op=Alu.max)
                v.tensor_mul(t["i"], t["s"][:, 0::2, :], t["s"][:, 1::2, :])
                v.tensor_mul(t["ap"], t["pwh4"][:, 0::2, :], t["pwh4"][:, 1::2, :])
                v.tensor_mul(t["at"], t["twh4"][:, 0::2, :], t["twh4"][:, 1::2, :])
                v.tensor_add(t["ap"], t["ap"], t["at"])             # A2
                v.tensor_scalar_add(t["i"], t["i"], 4e-4)
                v.tensor_mul(t["cr"], t["i"], t["ap"][:, ::-1, :])
                v.tensor_tensor(t["r"], t["cr"][:, 1, :], t["cr"][:, 0, :],
                                op=Alu.is_gt)
                v.tensor_tensor(t["n"], t["cr"][:, 0, :], t["cr"][:, 1, :],
                                op=Alu.is_gt)
                v.tensor_tensor(t["rn"], t["cr"][:, 0, :], t["cr"][:, 1, :],
                                op=Alu.not_equal)                   # r + n

            # ---- phase B: conf targets (GPSIMD) + xy_nr/fxy (DVE) ----
            for gi, t in enumerate(ts_):
                R = t["R"]
                # dc = pc0 + q, dna+1 = pc1 - q, with q = r*(pc1-pc0) - rn
                g_.tensor_sub(t["dpc"], t["pc2"][:, 1, :], t["pc2"][:, 0, :])
                g_.tensor_mul(t["dpc"], t["dpc"], t["r"])
                g_.tensor_sub(t["dpc"], t["dpc"], t["rn"])          # q
                g_.tensor_add(R[:, 8, :], t["pc2"][:, 0, :], t["dpc"])
                g_.tensor_sub(R[:, 9, :], t["pc2"][:, 1, :], t["dpc"])
                n_b2 = t["n"].unsqueeze(1).broadcast_to([P, 2, C])
                v.tensor_sub(t["d2"], t["txy4"][:, 2:4, :], t["txy4"][:, 0:2, :])
                v.tensor_mul(t["d2"], t["d2"], n_b2)
                v.tensor_add(t["d2"], t["txy4"][:, 0:2, :], t["d2"])  # xy_nr
                v.tensor_scalar_mul(t["u"], t["d2"], 7.0)
                # (u - 0.5005) + 1032 lands in [1024, 2048) where fp16 grain
                # is exactly 1.0 -> the fp16 store rounds to an integer.
                v.tensor_scalar(t["rt"], t["u"], 0.5005, 1032.0,
                                op0=Alu.subtract, op1=Alu.add)
                v.tensor_scalar(t["rt"], t["rt"], 1032.0, None,
                                op0=Alu.subtract)
                v.tensor_sub(t["u"], t["u"], t["rt"])               # fxy

            # ---- phase C: residuals + masks (DVE, ACT sqrt) + class diffs ----
            for gi, t in enumerate(ts_):
                R = t["R"]
                s.activation(t["sp"], t["pwh4"], Act.Sqrt, bias=eps_b)
                s.activation(t["st"], t["twh4"], Act.Sqrt, bias=eps_b)
                # R[0:8] = [xy0, wh0, xy1, wh1] residuals via [b, q] views
                Rb = R[:, 0:8, :].rearrange("p (b q) c -> p b q c", b=2)
                pxy_b = t["pxy4"].rearrange("p (b q) c -> p b q c", b=2)
                fxy_b = t["u"].unsqueeze(1).broadcast_to([P, 2, 2, C])
                v.tensor_sub(Rb[:, :, 0:2, :], pxy_b, fxy_b)
                sp_b = t["sp"].rearrange("p (b q) c -> p b q c", b=2)
                st_b = t["st"].rearrange("p (b q) c -> p b q c", b=2)
                v.tensor_sub(Rb[:, :, 2:4, :], sp_b, st_b)
                v.tensor_scalar(t["m0"], t["r"], -1.0, 1.0,
                                op0=Alu.mult, op1=Alu.add)
                m0_b4 = t["m0"].unsqueeze(1).broadcast_to([P, 4, C])
                r_b4 = t["r"].unsqueeze(1).broadcast_to([P, 4, C])
                v.tensor_mul(R[:, 0:4, :], R[:, 0:4, :], m0_b4)
                v.tensor_mul(R[:, 4:8, :], R[:, 4:8, :], r_b4)
                # GPSIMD wrote dc into R8 and dna+1 into R9
                v.tensor_scalar(R[:, 9, :], R[:, 9, :], 1.0, SQRTH,
                                op0=Alu.subtract, op1=Alu.mult)
                v.tensor_sub(R[:, 10:30, :], t["kl"][:, 0:20, :],
                             t["kl"][:, 20:40, :])

            # no-object cells: 0.5 * sum(pc^2)  (independent, fills ACT idle)
            s.activation(nob, nob, Act.Square, scale=SQRTH,
                         accum_out=acc[:, 6:7])

            # ---- phase D: square + accumulate (ACT) ----
            for gi, t in enumerate(ts_):
                R = t["R"]
                col = 3 * gi
                s.activation(R[:, 0:8, :], R[:, 0:8, :], Act.Square,
                             scale=SQRT5,
                             accum_out=acc[:, col : col + 1])
                s.activation(R[:, 8:10, :], R[:, 8:10, :], Act.Square,
                             accum_out=acc[:, col + 2 : col + 3])
                s.activation(R[:, 10:30, :], R[:, 10:30, :], Act.Square,
                             accum_out=acc[:, col + 1 : col + 2])

            nc.sync.dma_start(out=out_d[:], in_=acc[:])

    nc.compile()
    return nc


def _prep_core(all16: np.ndarray, obj_idx, non_idx, core: int):
    """all16: fp16 [NCELL, 60] = concat(pre, tgt) flattened per cell."""
    qo = (len(obj_idx) + NCORES - 1) // NCORES
    qn = (len(non_idx) + NCORES - 1) // NCORES
    assert qo <= CAP and qn <= CAPN, (qo, qn)
    oi = obj_idx[core * qo : (core + 1) * qo]
    ni = non_idx[core * qn : (core + 1) * qn]

    gob = all16[oi]
    box = np.empty((CAP, 18), dtype=np.float16)
    box[:] = PAD_ROW
    box[: len(oi)] = gob[:, BOX_CH]
    cls = np.zeros((CAP, 40), dtype=np.float16)
    cls[: len(oi)] = gob[:, CLS_CH]
    box = np.ascontiguousarray(
        box.reshape(G, P, C, 18).transpose(0, 1, 3, 2)
    ).reshape(G, P, 18 * C)
    cls = np.ascontiguousarray(
        cls.reshape(G, P, C, 40).transpose(0, 1, 3, 2)
    ).reshape(G, P, 40 * C)

    nob = np.zeros((CAPN, 2), dtype=np.float16)
    nob[: len(ni)] = all16[ni][:, [4, 9]]
    nob = np.ascontiguousarray(
        nob.reshape(P, CN, 2).transpose(0, 2, 1)
    ).reshape(P, 2 * CN)
    return {"box": box, "cls": cls, "nob": nob}


_NC_CACHE = None


def kernel(pre: np.ndarray, target: np.ndarray) -> np.ndarray:
    global _NC_CACHE
    if _NC_CACHE is None:
        _NC_CACHE = _build()
    nc = _NC_CACHE

    pre3 = np.asarray(pre, dtype=np.float32).reshape(NCELL, 30)
    tgt3 = np.asarray(target, dtype=np.float32).reshape(NCELL, 30)
    objmask = tgt3[:, 4] > 0
    all16 = np.concatenate(
        [pre3.astype(np.float16), tgt3.astype(np.float16)], axis=1
    )
    obj_idx = np.flatnonzero(objmask)
    non_idx = np.flatnonzero(~objmask)

    in_maps = [
        _prep_core(all16, obj_idx, non_idx, i) for i in range(NCORES)
    ]
    res = run_bass_kernel_spmd(nc, in_maps, core_ids=list(range(NCORES)))
    total = 0.0
    for r in res.results:
        total += float(np.sum(r["out"].astype(np.float64)))
    return np.float32(total / B)
